# revision 14
# baseline (speedup 1.0000x reference)
"""BSplineKan layer kernel for 8 trn2 NeuronCores.

Math: out[b,o] = w_b*sum_i silu(x[b,i]) + w_s*sum_{i,k} bases_k(x[b,i]) * P[o,i,k]
with quadratic B-spline bases on 16 uniform knots over [-1.125, 1.125] and
x ~ U[0,1).

Only bases k=5..12 are nonzero for x in [0,1). On uniform knots the spline
telescopes into truncated-power features:
    B2(t) = 0.5*[r(t) - 3r(t-1) + 3r(t-2) - r(t-3)],  r(t) = relu(t)^2
Folding this per-(o,i) into host-precomputed weights, the device computes 9
feature planes per input element:
    v = x-1/2, v^2, relu^2(c_j - x) for the 3 interior knots left of 1/2,
    relu^2(x - c_j) for the 4 right of 1/2
(left-facing knots + centering keep plane magnitudes balanced, which makes
fp32r's 11-bit-mantissa rounding benign), followed by one fp32r matmul with
K = 9*I. The constant term becomes a host-side bias. The silu sum is a
separate ACT pass over natural-layout x using accum_out, shipped to the host
as a tiny per-(core,b) column.

Sharding: contraction split — core c owns i in [128c, 128c+128). Each core
emits partial (2048, 1024) outputs in fp16; the host sums the 8 partials in
fp64 and adds bias + w_b * silu. No device collectives.
"""

import numpy as np

import concourse.bass as bass
import concourse.bass_utils as _bu
import concourse.mybir as mybir
import concourse.tile as tile
from concourse import bacc
from concourse.bass_utils import run_bass_kernel_spmd

F32 = mybir.dt.float32
F32R = mybir.dt.float32r
F16 = mybir.dt.float16
AF = mybir.ActivationFunctionType
ALU = mybir.AluOpType

B, I, O = 2048, 1024, 1024
N_CORES = 8
I_LOC = I // N_CORES       # 128 contraction rows per core
H = 2.25 / 15.0            # knot spacing 0.15
KNOTS = [j * H - 1.125 for j in range(8, 15)]   # interior knots in (0,1)
LEFT = KNOTS[:3]           # 0.075 0.225 0.375  -> relu^2(c - x)
RIGHT = KNOTS[3:]          # 0.525 0.675 0.825 0.975 -> relu^2(x - c)
N_PLANES = 9               # v, v^2, 3 left, 4 right
N_TB = B // 128            # 16 batch tiles
N_OC = O // 512            # 2 output chunks of 512 (fp32 moving-dim max)

# enable walrus ldw-opt (pipelines fp32r weight loads behind streaming;
# default-off flag hardcoded in bass_utils).
_orig_run_command = _bu.run_command


def _run_command_ldwopt(argv, **kwargs):
    argv = ["--enable-ldw-opt=true" if a == "--enable-ldw-opt=false" else a
            for a in argv]
    return _orig_run_command(argv, **kwargs)


def _round_fp32r(a: np.ndarray) -> np.ndarray:
    """Round-to-nearest fp32 -> fp32r (11-bit mantissa, low 12 bits zero)."""
    u = np.ascontiguousarray(a, np.float32).view(np.uint32)
    u = (u + np.uint32(0x800)) & np.uint32(0xFFFFF000)
    return u.view(np.float32)


def fold_weights(P: np.ndarray, w_s: float):
    """Fold spline parameters into per-plane weights.

    Returns W (N_PLANES, I, O) float32 (fp32r-rounded) and bias (O,) float64.
    """
    Pd = P.astype(np.float64)
    O_, I_, _ = P.shape
    # G_j = coefficient of r_j = relu^2(u - j), u = (x + 1.125)/H, j = 5..14
    Pz = np.zeros((O_, I_, 18))
    Pz[:, :, 5:13] = Pd[:, :, 5:13]
    G = np.zeros((O_, I_, 15))
    for j in range(5, 15):
        G[:, :, j] = (0.5 * Pz[:, :, j] - 1.5 * Pz[:, :, j - 1]
                      + 1.5 * Pz[:, :, j - 2] - 0.5 * Pz[:, :, j - 3])
    c = np.array([j * H - 1.125 for j in range(15)])
    inv_h2 = 1.0 / (H * H)
    # ungated j=5,6,7 (u >= 7.5 always): (x - c_j)^2 / H^2 -> quadratic in x
    A = (G[:, :, 5] + G[:, :, 6] + G[:, :, 7]) * inv_h2
    Bq = -2.0 * (c[5] * G[:, :, 5] + c[6] * G[:, :, 6] + c[7] * G[:, :, 7]) * inv_h2
    Cq = (c[5] ** 2 * G[:, :, 5] + c[6] ** 2 * G[:, :, 6] + c[7] ** 2 * G[:, :, 7]) * inv_h2
    D = [G[:, :, 8 + t] * inv_h2 for t in range(7)]  # gated knots, x-units
    # flip left-of-center knots: D*relu^2(x-c) = D*(x-c)^2 - D*relu^2(c-x)
    left_w = []
    for t, cj in enumerate(LEFT):
        A += D[t]
        Bq += -2.0 * cj * D[t]
        Cq += cj * cj * D[t]
        left_w.append(-D[t])
    right_w = [D[3 + t] for t in range(4)]
    # recenter the quadratic at 1/2: A x^2 + B x + C = A v^2 + (A+B) v + const
    planes = [Bq + A, A] + left_w + right_w                   # each (O, I)
    bias = (Cq + 0.5 * Bq + 0.25 * A).sum(axis=1) * w_s       # (O,)
    W = np.empty((N_PLANES, I_, O_), np.float32)
    for p, pw in enumerate(planes):
        W[p] = (w_s * pw).T.astype(np.float32)
    return _round_fp32r(W), bias


def build_kernel(reps: int = 1, chunk_feat: int = 1, w_split: str = 'half',
                 dve_squares: tuple = (), order: str = 'tb', out_batch: int = 1,
                 n_ps: int = 4):
    """Per-core Bass kernel (SPMD across 8 cores, contraction-split).

    reps > 1 wraps the body in a hardware loop for timing runs.
    """
    nc = bacc.Bacc("TRN2", target_bir_lowering=False, debug=False,
                   num_devices=N_CORES)
    xT_d = nc.dram_tensor("xT", [I_LOC, B], F32, kind="ExternalInput")
    xN_d = nc.dram_tensor("xN", [B, I_LOC], F32, kind="ExternalInput")
    W_d = nc.dram_tensor("Wf", [N_PLANES * I_LOC, O], F32R, kind="ExternalInput")
    y_d = nc.dram_tensor("y", [B, O], F16, kind="ExternalOutput")
    s_d = nc.dram_tensor("ysilu", [128, N_TB], F32, kind="ExternalOutput")

    with tile.TileContext(nc) as tc:
        with (
            tc.tile_pool(name="xp", bufs=1) as x_pool,
            tc.tile_pool(name="wp", bufs=1) as w_pool,
            tc.tile_pool(name="fp", bufs=1) as f_pool,
            tc.tile_pool(name="sp", bufs=2) as s_pool,
            tc.tile_pool(name="op", bufs=4) as o_pool,
            tc.tile_pool(name="cp", bufs=1) as c_pool,
            tc.tile_pool(name="ps", bufs=1, space="PSUM") as ps_pool,
        ):
            consts = c_pool.tile([128, 1], F32, name="consts")
            nc.vector.memset(consts[:, 0:1], -0.5)

            def body(_iv=None):
                # W split per plane across both HWDGE queues so plane j is
                # resident by the time the first psum group's j-th matmul
                # issues
                wt = w_pool.tile([128, N_PLANES * O], F32R, name="wt")
                src = W_d[:].rearrange("(j p) o -> p j o", p=128)
                if w_split == 'plane':
                    for j in range(N_PLANES):
                        eng = nc.sync if j % 2 == 0 else nc.scalar
                        eng.dma_start(wt[:, j * O:(j + 1) * O], src[:, j, :])
                else:
                    w3 = wt[:].rearrange("p (j o) -> p j o", j=N_PLANES)
                    nc.sync.dma_start(w3[:, 0:4, :], src[:, 0:4, :])
                    nc.scalar.dma_start(w3[:, 4:N_PLANES, :], src[:, 4:N_PLANES, :])
                xt = x_pool.tile([128, B], F32, name="xt")
                CH = chunk_feat             # feature chunking along batch
                BC = B // CH
                for ch in range(CH):
                    eng = nc.sync if ch % 2 == 0 else nc.scalar
                    eng.dma_start(xt[:, ch * BC:(ch + 1) * BC],
                                  xT_d[:, ch * BC:(ch + 1) * BC])
                xn = x_pool.tile([128, N_TB * I_LOC], F32, name="xn")
                nc.scalar.dma_start(
                    xn[:].rearrange("p (t i) -> p t i", t=N_TB),
                    xN_d[:].rearrange("(t p) i -> p t i", p=128))

                ft = f_pool.tile([128, N_PLANES * B], F32R, name="ft")

                def plane(p, ch):
                    return ft[:, p * B + ch * BC:p * B + (ch + 1) * BC]

                # chunk-major production so early batch tiles unblock matmuls
                # p0: v = x - 1/2 (DVE tensor_scalar, 2x fp32 mode)
                # p1: v^2 (ACT square with bias)
                # p2-4: relu^2(c - x) via min(x - c, 0) then square
                # p5-8: relu^2(x - c) via max(x - c, 0) then square
                for ch in range(CH):
                    xs = xt[:, ch * BC:(ch + 1) * BC]
                    nc.vector.tensor_scalar(plane(0, ch), xs, 0.5, None,
                                            ALU.subtract)
                    nc.scalar.activation(plane(1, ch), xs, AF.Square,
                                         bias=consts[:, 0:1], scale=1.0)
                    for t, cj in enumerate(LEFT + RIGHT):
                        gate = ALU.min if t < 3 else ALU.max
                        r = s_pool.tile([128, BC], F32, tag="r", name=f"r{ch}_{t}")
                        nc.vector.tensor_scalar(r[:], xs, float(cj), 0.0,
                                                ALU.subtract, gate)
                        if t in dve_squares:
                            nc.vector.tensor_tensor(plane(2 + t, ch), r[:], r[:],
                                                    ALU.mult)
                        else:
                            nc.scalar.activation(plane(2 + t, ch), r[:], AF.Square)

                # silu sum over this core's i-slice, per batch row: ACT pass
                # on natural-layout x with accum_out
                acc = c_pool.tile([128, N_TB], F32, name="acc")
                for tb in range(N_TB):
                    sil = s_pool.tile([128, I_LOC], F32, tag="sil", name=f"sil{tb}")
                    nc.scalar.activation(
                        sil[:], xn[:, tb * I_LOC:(tb + 1) * I_LOC], AF.Silu,
                        accum_out=acc[:, tb:tb + 1])
                nc.sync.dma_start(s_d[:], acc[:])

                if order == 'tb':
                    if out_batch > 1:
                        ot_big = x_pool.tile([128, N_TB * O], F16, name="otb")
                        y3 = y_d[:].rearrange("(t p) o -> p t o", p=128)
                        o3 = ot_big[:].rearrange("p (t o) -> p t o", t=N_TB)
                    for tb in range(N_TB):
                        if out_batch == 1:
                            ot = o_pool.tile([128, O], F16, tag="ot", name=f"ot{tb}")
                        for oc in range(N_OC):
                            g = tb * N_OC + oc
                            ps = ps_pool.tile([128, 512], F32, tag=f"ps{g % n_ps}",
                                              name=f"ps{tb}_{oc}")
                            for j in range(N_PLANES):
                                nc.tensor.matmul(
                                    ps[:],
                                    ft[:, j * B + tb * 128:j * B + (tb + 1) * 128],
                                    wt[:, j * O + oc * 512:j * O + oc * 512 + 512],
                                    start=(j == 0), stop=(j == N_PLANES - 1),
                                )
                            dst = (ot[:, oc * 512:(oc + 1) * 512] if out_batch == 1
                                   else ot_big[:, tb * O + oc * 512:tb * O + (oc + 1) * 512])
                            if g % 2 == 0:
                                nc.vector.tensor_copy(dst, ps[:])
                            else:
                                nc.scalar.copy(dst, ps[:])
                        if out_batch == 1:
                            eng = nc.sync if tb % 2 == 0 else nc.scalar
                            eng.dma_start(y_d[tb * 128:(tb + 1) * 128, :], ot[:])
                        elif tb % out_batch == out_batch - 1:
                            blk = tb // out_batch
                            eng = nc.sync if blk % 2 == 0 else nc.scalar
                            eng.dma_start(
                                y3[:, blk * out_batch:(blk + 1) * out_batch, :],
                                o3[:, blk * out_batch:(blk + 1) * out_batch, :])
                elif order == 'burst3':
                    # 3-plane bursts per bank dwell: overlaps feature
                    # production at 3-plane granularity while keeping PSUM
                    # bank switches 3x rarer than pure plane-major
                    GPB = 8
                    for sb in range(32 // GPB):
                        pst = {}
                        for jt in (0, 3, 6):
                            nj = 3 if jt < 6 else N_PLANES - 6
                            for g in range(sb * GPB, (sb + 1) * GPB):
                                tb, oc = g // N_OC, g % N_OC
                                if jt == 0:
                                    pst[g] = ps_pool.tile(
                                        [128, 512], F32, tag=f"ps{g % GPB}",
                                        name=f"ps{tb}_{oc}")
                                for j in range(jt, jt + nj):
                                    nc.tensor.matmul(
                                        pst[g][:],
                                        ft[:, j * B + tb * 128:j * B + (tb + 1) * 128],
                                        wt[:, j * O + oc * 512:j * O + oc * 512 + 512],
                                        start=(j == 0), stop=(j == N_PLANES - 1),
                                    )
                        ots = {}
                        for g in range(sb * GPB, (sb + 1) * GPB):
                            tb, oc = g // N_OC, g % N_OC
                            if oc == 0:
                                ots[tb] = o_pool.tile([128, O], F16, tag="ot",
                                                      name=f"ot{tb}")
                            if g % 2 == 0:
                                nc.vector.tensor_copy(
                                    ots[tb][:, oc * 512:(oc + 1) * 512], pst[g][:])
                            else:
                                nc.scalar.copy(
                                    ots[tb][:, oc * 512:(oc + 1) * 512], pst[g][:])
                        for tb in sorted(ots):
                            eng = nc.sync if tb % 2 == 0 else nc.scalar
                            eng.dma_start(y_d[tb * 128:(tb + 1) * 128, :], ots[tb][:])
                else:
                    # plane-major sweeps over 8 concurrently-open PSUM groups:
                    # PE streams plane j across 8 groups (~1.7us) while ACT
                    # produces plane j+1 (~1.9us) -> production pipelines
                    # under the matmuls instead of stalling the first group
                    GPB = 8                       # groups per super-batch
                    for sb in range(32 // GPB):
                        pst = {}
                        for j in range(N_PLANES):
                            for g in range(sb * GPB, (sb + 1) * GPB):
                                tb, oc = g // N_OC, g % N_OC
                                if j == 0:
                                    pst[g] = ps_pool.tile(
                                        [128, 512], F32, tag=f"ps{g % GPB}",
                                        name=f"ps{tb}_{oc}")
                                nc.tensor.matmul(
                                    pst[g][:],
                                    ft[:, j * B + tb * 128:j * B + (tb + 1) * 128],
                                    wt[:, j * O + oc * 512:j * O + oc * 512 + 512],
                                    start=(j == 0), stop=(j == N_PLANES - 1),
                                )
                        ots = {}
                        for g in range(sb * GPB, (sb + 1) * GPB):
                            tb, oc = g // N_OC, g % N_OC
                            if oc == 0:
                                ots[tb] = o_pool.tile([128, O], F16, tag="ot",
                                                      name=f"ot{tb}")
                            if g % 2 == 0:
                                nc.vector.tensor_copy(
                                    ots[tb][:, oc * 512:(oc + 1) * 512], pst[g][:])
                            else:
                                nc.scalar.copy(
                                    ots[tb][:, oc * 512:(oc + 1) * 512], pst[g][:])
                        for tb in sorted(ots):
                            eng = nc.sync if tb % 2 == 0 else nc.scalar
                            eng.dma_start(y_d[tb * 128:(tb + 1) * 128, :], ots[tb][:])

            if reps == 1:
                body()
            else:
                with tc.For_i(0, reps, 1) as iv:
                    body(iv)
    nc.compile()
    return nc


_cached_nc = None


def _get_nc():
    global _cached_nc
    if _cached_nc is None:
        _bu.run_command = _run_command_ldwopt
        _cached_nc = build_kernel(reps=1)
    return _cached_nc


def prepare_inputs(x, spline_parameters, w_b, w_s):
    """Host-side prep: returns (in_maps, bias, w_b) for the 8 cores."""
    x = np.ascontiguousarray(np.asarray(x, np.float32))
    P = np.asarray(spline_parameters, np.float32)
    w_b = float(np.asarray(w_b))
    W, bias = fold_weights(P, float(np.asarray(w_s)))
    xT = np.ascontiguousarray(x.T)                     # (I, B)
    in_maps = []
    for c in range(N_CORES):
        sl = slice(c * I_LOC, (c + 1) * I_LOC)
        in_maps.append({
            "xT": np.ascontiguousarray(xT[sl, :]),
            "xN": np.ascontiguousarray(x[:, sl]),
            "Wf": np.ascontiguousarray(
                W[:, sl, :].reshape(N_PLANES * I_LOC, O)),
        })
    return in_maps, bias, w_b


def kernel(x, spline_parameters, w_b, w_s):
    in_maps, bias, w_b = prepare_inputs(x, spline_parameters, w_b, w_s)
    nc = _get_nc()
    res = run_bass_kernel_spmd(nc, in_maps, core_ids=list(range(N_CORES)))
    acc = np.zeros((B, O), np.float64)
    silu_sum = np.zeros((B,), np.float64)
    for c in range(N_CORES):
        acc += res.results[c]["y"].astype(np.float64)
        # ysilu[p, t] holds sum_i silu(x[t*128+p, i_slice])
        silu_sum += res.results[c]["ysilu"].T.reshape(B)
    acc += bias[None, :]
    acc += (w_b * silu_sum)[:, None]
    return acc.astype(np.float32)
